# revision 1
# baseline (speedup 1.0000x reference)
"""EdgeConv (PyG, aggr='max') Trainium2 kernel, 8-core SPMD.

Math: out_i = max_{e: dst(e)=i} relu(x_i @ W1.T + (x_src(e) - x_i) @ W2.T + b)
with W = [W1 | W2].  Rewriting:
    msg_e = relu(A_i + g_src(e)),  A = x @ (W1-W2).T + b,  g = x @ W2.T
Since A_i is constant within segment i and relu is monotone:
    out_i = relu(A_i + max_e g_src(e))
The reference's dst is repeat(arange(N), DEG) (fixed-degree kNN-style graph),
so segments are 16 consecutive edges; segment-max becomes a grouped reduce.

Two SPMD launches on 8 cores:
  L1 (node-parallel): per-core 6250-node shard computes A (row-major f32)
     and g (row-major bf16) via PE transpose of x tiles + one matmul.
  L2 (edge-parallel): per-core 100k-edge shard bulk-gathers 256B bf16
     row-PAIRS [g_{2r} | g_{2r+1}] by src>>1 with non-transpose dma_gather
     (one descriptor per edge — half the HBM bytes of fp32 pairs; src>>1 <=
     24999 fits int16 with no table split; pad positions read the sentinel
     pair-row NPAIR = -3e38). The parity half-select uses a host-precomputed
     uint8 mask via copy_predicated (plain copy on ACT, predicated overwrite
     on DVE). A host-side edge permutation lands node n's 16 slots at
     partition n%128, slots 16*(n//128)+k, so the segment max is a free-dim
     grouped reduce emitting f32 row-major directly.
"""

import numpy as np

N_NODES = 50000
DEG = 16
C = 64
N_CORES = 8
NSH = N_NODES // N_CORES  # 6250 nodes per core
P = 128
TCH = 2  # node-tiles per L2 chunk
CHUNK = TCH * P  # 256 nodes per L2 chunk
NSH_PAD = 6400  # 25 chunks * 256; 50 tiles * 128
NT = NSH_PAD // P  # 50
NCHUNKS = NSH_PAD // CHUNK  # 25
NI = CHUNK * DEG  # 4096 gather positions per chunk
NPAIR = N_NODES // 2  # 512B row-pairs in the gather table
SENT = -3.0e38

_cache = {}


def _build_dense():
    import concourse.bacc as bacc
    import concourse.mybir as mybir
    from concourse.tile import TileContext
    from concourse.masks import make_identity

    nc = bacc.Bacc("TRN2", target_bir_lowering=False, debug=False)
    f32 = mybir.dt.float32
    bf16 = mybir.dt.bfloat16
    xs = nc.dram_tensor("xs", [NSH_PAD, C], f32, kind="ExternalInput")
    w = nc.dram_tensor("w", [C, 2 * C], f32, kind="ExternalInput")
    bb = nc.dram_tensor("bb", [P, C], f32, kind="ExternalInput")
    gout = nc.dram_tensor("gout", [NSH_PAD, C], bf16, kind="ExternalOutput")
    aout = nc.dram_tensor("aout", [NSH_PAD, C], f32, kind="ExternalOutput")

    with TileContext(nc) as tc:
        with (
            tc.tile_pool(name="const", bufs=1) as cpool,
            tc.tile_pool(name="sbuf", bufs=4) as pool,
            tc.tile_pool(name="psum", bufs=2, space="PSUM") as psum,
        ):
            ident = cpool.tile([P, P], f32)
            make_identity(nc, ident[:])
            w_sb = cpool.tile([C, 2 * C], f32)
            nc.sync.dma_start(out=w_sb[:], in_=w[:])
            bb_sb = cpool.tile([P, C], f32)
            nc.sync.dma_start(out=bb_sb[:], in_=bb[:])

            # wcat = [V1.T | V2.T] ([64, 128]): V1=W1-W2, V2=W2.
            t1 = psum.tile([C, C], f32, tag="wt")
            t2 = psum.tile([C, C], f32, tag="wt")
            nc.tensor.transpose(out=t1[:], in_=w_sb[:, 0:C], identity=ident[:C, :C])
            nc.tensor.transpose(out=t2[:], in_=w_sb[:, C : 2 * C], identity=ident[:C, :C])
            wcat = cpool.tile([C, 2 * C], f32)
            nc.vector.tensor_copy(out=wcat[:, C : 2 * C], in_=t2[:])
            nc.vector.tensor_sub(out=wcat[:, 0:C], in0=t1[:], in1=wcat[:, C : 2 * C])

            G = 5  # node-tiles per supertile (NT == 50 == 10 * G)
            for st in range(NT // G):
                rows = slice(st * G * P, (st + 1) * G * P)
                xt = pool.tile([P, G, C], f32, tag="xt")
                nc.sync.dma_start(
                    out=xt[:], in_=xs[rows, :].rearrange("(g p) c -> p g c", p=P)
                )
                g_sup = pool.tile([P, G, C], bf16, tag="g")
                a_sup = pool.tile([P, G, C], f32, tag="a")
                for g in range(G):
                    xT_ps = psum.tile([C, P], f32, tag="xT")
                    nc.tensor.transpose(
                        out=xT_ps[:], in_=xt[:, g, :], identity=ident[:]
                    )
                    xT = pool.tile([C, P], f32, tag="xTs")
                    nc.vector.tensor_copy(out=xT[:], in_=xT_ps[:])
                    h_ps = psum.tile([P, 2 * C], f32, tag="h")
                    nc.tensor.matmul(
                        out=h_ps[:], lhsT=xT[:], rhs=wcat[:], start=True, stop=True
                    )
                    nc.scalar.copy(out=g_sup[:, g, :], in_=h_ps[:, C : 2 * C])
                    nc.vector.tensor_add(
                        out=a_sup[:, g, :], in0=h_ps[:, 0:C], in1=bb_sb[:]
                    )
                nc.sync.dma_start(
                    out=gout[rows, :].rearrange("(g p) c -> p g c", p=P), in_=g_sup[:]
                )
                nc.sync.dma_start(
                    out=aout[rows, :].rearrange("(g p) c -> p g c", p=P), in_=a_sup[:]
                )
    nc.compile()
    return nc


def _build_gather():
    import concourse.bacc as bacc
    import concourse.mybir as mybir
    from concourse.tile import TileContext

    nc = bacc.Bacc("TRN2", target_bir_lowering=False, debug=False)
    f32 = mybir.dt.float32
    bf16 = mybir.dt.bfloat16
    i16 = mybir.dt.int16
    # pair table: row r = [g_{2r} | g_{2r+1}] (512B); row NPAIR = sentinel
    gpair = nc.dram_tensor("gpair", [NPAIR + 1, 2 * C], bf16, kind="ExternalInput")
    idx = nc.dram_tensor("idx", [NCHUNKS * P, NI // 16], i16, kind="ExternalInput")
    msk = nc.dram_tensor("msk", [NCHUNKS * P, TCH * DEG], mybir.dt.uint8, kind="ExternalInput")
    ash = nc.dram_tensor("ash", [NSH_PAD, C], f32, kind="ExternalInput")
    osh = nc.dram_tensor("osh", [NSH_PAD, C], f32, kind="ExternalOutput")

    with TileContext(nc) as tc:
        with (
            tc.tile_pool(name="sbuf", bufs=4) as pool,
            tc.tile_pool(name="gat", bufs=4) as gpool,
        ):
            idx_all = pool.tile([P, NCHUNKS, NI // 16], i16, bufs=1)
            nc.sync.dma_start(
                out=idx_all[:], in_=idx[:, :].rearrange("(h p) s -> p h s", p=P)
            )
            msk_all = pool.tile([P, NCHUNKS, TCH * DEG], mybir.dt.uint8, bufs=1)
            nc.sync.dma_start(
                out=msk_all[:], in_=msk[:, :].rearrange("(h p) s -> p h s", p=P)
            )
            a_all = pool.tile([P, NT, C], f32, bufs=1)
            nc.sync.dma_start(
                out=a_all[:], in_=ash[:, :].rearrange("(t p) c -> p t c", p=P)
            )
            for ch in range(NCHUNKS):
                ncols = slice(ch * CHUNK, (ch + 1) * CHUNK)
                # position j lands at partition j%128, slot j//128; each slot
                # holds a 512B row-pair [even | odd]
                gath = gpool.tile([P, TCH * DEG, 2 * C], bf16, tag="gath")
                nc.gpsimd.dma_gather(
                    out_ap=gath[:],
                    in_ap=gpair[:],
                    idxs_ap=idx_all[:, ch, :],
                    num_idxs=NI,
                    num_idxs_reg=NI,
                    elem_size=2 * C,
                    transpose=False,
                    queue_num=0,
                    single_packet=False,
                )
                # select the parity half: even by default (ACT), odd where mask=1
                sel = gpool.tile([P, TCH * DEG, C], bf16, tag="sel")
                nc.scalar.copy(out=sel[:], in_=gath[:, :, 0:C])
                nc.vector.copy_predicated(
                    out=sel[:],
                    mask=msk_all[:, ch, :].to_broadcast([P, TCH * DEG, C]),
                    data=gath[:, :, C : 2 * C],
                )
                m_sb = pool.tile([P, TCH, C], f32, tag="m")
                nc.vector.tensor_reduce(
                    out=m_sb[:],
                    in_=sel[:].rearrange("p (t k) c -> p t c k", k=DEG),
                    axis=mybir.AxisListType.X,
                    op=mybir.AluOpType.max,
                )
                s_sb = pool.tile([P, TCH, C], f32, tag="s")
                nc.vector.tensor_add(
                    out=s_sb[:], in0=m_sb[:], in1=a_all[:, ch * TCH : (ch + 1) * TCH, :]
                )
                o_sb = pool.tile([P, TCH, C], f32, tag="o")
                nc.scalar.activation(
                    out=o_sb[:], in_=s_sb[:], func=mybir.ActivationFunctionType.Relu
                )
                nc.sync.dma_start(
                    out=osh[ncols, :].rearrange("(t p) c -> p t c", p=P), in_=o_sb[:]
                )
    nc.compile()
    return nc


def _make_indices(src_pad):
    """src_pad: [NSH_PAD, DEG] int64 node ids (pad rows = -1).
    Returns (idx, msk): pair-row indices (src>>1, sentinel NPAIR for pads) in
    dma_gather's index layout, and the odd-parity mask in dest layout
    [128, slots]. Position j of chunk ch covers node n_c = j%128 + 128*(j//128
    // DEG) ... specifically j = (DEG*(n_c//128)+k)*128 + (n_c%128)."""
    s = src_pad.reshape(NCHUNKS, TCH, P, DEG)
    flat = np.transpose(s, (0, 1, 3, 2)).reshape(NCHUNKS, NI)  # [ch, (t k p)]
    pidx = np.where(flat >= 0, flat >> 1, NPAIR).astype(np.int16)
    par = np.where(flat >= 0, flat & 1, 0).astype(np.uint8)
    # index layout: position j -> [j%16, j//16], replicated 8x down partitions
    a = np.swapaxes(pidx.reshape(NCHUNKS, NI // 16, 16), 1, 2)
    idx = np.ascontiguousarray(
        np.tile(a, (1, 8, 1)).reshape(NCHUNKS * P, NI // 16)
    )
    # mask layout: dest [partition j%128, slot j//128]
    m = np.swapaxes(par.reshape(NCHUNKS, TCH * DEG, P), 1, 2)
    msk = np.ascontiguousarray(m.reshape(NCHUNKS * P, TCH * DEG))
    return idx, msk


def _numpy_fallback(x, edge_index, W, b):
    src, dst = edge_index[0], edge_index[1]
    V1 = W[:, :C] - W[:, C:]
    V2 = W[:, C:]
    A = x @ V1.T + b
    g = x @ V2.T
    out = np.full((x.shape[0], C), -np.inf, dtype=np.float32)
    msg = np.maximum(A[dst] + g[src], 0.0)
    np.maximum.at(out, dst, msg)
    return np.where(np.isneginf(out), 0.0, out).astype(np.float32)


def _run_spmd(nc, in_maps):
    # the shared axon device occasionally reports a transient
    # NRT_EXEC_UNIT_UNRECOVERABLE on a cold first launch; retry once
    import time
    from concourse.bass_utils import run_bass_kernel_spmd

    try:
        return run_bass_kernel_spmd(nc, in_maps, core_ids=list(range(N_CORES)))
    except Exception:
        time.sleep(10.0)
        return run_bass_kernel_spmd(nc, in_maps, core_ids=list(range(N_CORES)))


def kernel(x, edge_index, edge_attr, W, b):

    x = np.ascontiguousarray(x, dtype=np.float32)
    edge_index = np.ascontiguousarray(edge_index, dtype=np.int32)
    W = np.ascontiguousarray(W, dtype=np.float32)
    b = np.ascontiguousarray(b, dtype=np.float32)

    expected_dst = np.repeat(np.arange(N_NODES, dtype=np.int32), DEG)
    if (
        x.shape != (N_NODES, C)
        or edge_index.shape != (2, N_NODES * DEG)
        or not np.array_equal(edge_index[1], expected_dst)
    ):
        return _numpy_fallback(x, edge_index, W, b)

    if "dense" not in _cache:
        _cache["dense"] = _build_dense()
    if "gather" not in _cache:
        _cache["gather"] = _build_gather()

    # ---- Launch 1: node-parallel dense phase ----
    bb = np.ascontiguousarray(np.broadcast_to(b, (P, C)))
    in1 = []
    for c in range(N_CORES):
        xs = np.zeros((NSH_PAD, C), dtype=np.float32)
        xs[:NSH] = x[c * NSH : (c + 1) * NSH]
        in1.append({"xs": xs, "w": W, "bb": bb})
    r1 = _run_spmd(_cache["dense"], in1)

    g_full = np.concatenate(
        [r1.results[c]["gout"][:NSH] for c in range(N_CORES)], axis=0
    )
    gpair = np.concatenate(
        [g_full.reshape(NPAIR, 2 * C), np.full((1, 2 * C), SENT, dtype=g_full.dtype)],
        axis=0,
    )
    gpair = np.ascontiguousarray(gpair)

    # ---- Launch 2: edge-parallel gather + segment max ----
    src = edge_index[0]
    in2 = []
    for c in range(N_CORES):
        s = np.full((NSH_PAD, DEG), -1, dtype=np.int64)
        s[:NSH] = src[c * NSH * DEG : (c + 1) * NSH * DEG].reshape(NSH, DEG)
        idx, msk = _make_indices(s)
        in2.append(
            {"gpair": gpair, "idx": idx, "msk": msk, "ash": r1.results[c]["aout"]}
        )
    r2 = _run_spmd(_cache["gather"], in2)

    out = np.concatenate(
        [r2.results[c]["osh"][:NSH] for c in range(N_CORES)], axis=0
    ).astype(np.float32)
    _cache["last_results"] = (r1, r2)
    return out



# revision 20
# speedup vs baseline: 1.2126x; 1.2126x over previous
"""EdgeConv (PyG, aggr='max') Trainium2 kernel, 8-core SPMD.

Math: out_i = max_{e: dst(e)=i} relu(x_i @ W1.T + (x_src(e) - x_i) @ W2.T + b)
with W = [W1 | W2].  Rewriting:
    msg_e = relu(A_i + g_src(e)),  A = x @ (W1-W2).T + b,  g = x @ W2.T
Since A_i is constant within segment i and relu is monotone:
    out_i = relu(A_i + max_e g_src(e))
The reference's dst is repeat(arange(N), DEG) (fixed-degree graph), so
segments are 16 consecutive edges; segment-max becomes a grouped reduce.

Two SPMD launches on 8 cores (dst-sharded, 6250 nodes/core):

L1 (dense): host supplies the core's x shard TRANSPOSED in bf16 with an
   appended ones row ([65, 6400], columns in pair-major tile order), so
   g = x @ W2.T needs no on-device transposes: 50 direct matmuls
   (lhsT = xT tile) writing the bf16 pair table slice [3200, 128]
   (row r = [g_2r | g_2r+1]).

L2 (gather): per-core 100k-edge shard bulk-gathers 256B bf16 row-PAIRS
   by src>>1 with dma_gather (one descriptor per edge; src>>1 <= 24999
   fits int16; pad slots read sentinel pair-row 25000 = -3e38), batched
   2 node-chunks (8192 descriptors) per gather. Parity is resolved by an
   in-place copy_predicated (odd rows overwrite the even half where the
   host-built mask is 1), then a 4-round pairwise tree-max reduces each
   segment's 16 slots. A = x @ (W1-W2).T + b is recomputed from xT on
   device (cheaper than an HBM round-trip; bias via the ones row).
   Node layout is pair-major (node n -> partition (n//2)%128, tile
   (n//2)//128, half n%2) so every bulk DMA moves >=256B contiguous
   runs and the output store hits 512B descriptors.
"""

import numpy as np

N_NODES = 50000
DEG = 16
C = 64
N_CORES = 8
NSH = N_NODES // N_CORES  # 6250 nodes per core
P = 128
NSH_PAD = 6400  # 25 tiles * 256 nodes (pair-major: tile = 128 pairs)
NT = 25  # node tiles (256 nodes each) per core
NPAIR = N_NODES // 2  # 25000 pair rows in the global gather table
SENT = -3.0e38
# gather batches (tile0, ntiles): 2-tile batches, then 1-tile ones so the
# post-gather DVE tail drains quickly
BATCHES = [(2 * i, 2) for i in range(11)] + [(22, 1), (23, 1), (24, 1)]
NB = len(BATCHES)
IDXW = NSH_PAD * DEG // 16  # 6400 int16 idx words per partition

_cache = {}


def _bf16():
    import ml_dtypes

    return np.dtype(ml_dtypes.bfloat16)


def _build_dense():
    import concourse.bacc as bacc
    import concourse.mybir as mybir
    from concourse.tile import TileContext
    from concourse.masks import make_identity

    nc = bacc.Bacc("TRN2", target_bir_lowering=False, debug=False)
    f32 = mybir.dt.float32
    bf16 = mybir.dt.bfloat16
    xta = nc.dram_tensor("xta", [C + 1, NSH_PAD], bf16, kind="ExternalInput")
    w = nc.dram_tensor("w", [C, 2 * C], f32, kind="ExternalInput")
    gp = nc.dram_tensor("gp", [NSH_PAD // 2, 2 * C], bf16, kind="ExternalOutput")

    with TileContext(nc) as tc:
        with (
            tc.tile_pool(name="const", bufs=1) as cpool,
            tc.tile_pool(name="sbuf", bufs=2) as pool,
            tc.tile_pool(name="psum", bufs=3, space="PSUM") as psum,
        ):
            xt = cpool.tile([C, NT, 2, P], bf16)
            xv = xta[0:C, :].rearrange("c (t s p) -> c t s p", s=2, p=P)
            nc.sync.dma_start(out=xt[:, 0:2, :, :], in_=xv[:, 0:2, :, :])
            nc.sync.dma_start(out=xt[:, 2:, :, :], in_=xv[:, 2:, :, :])
            ident = cpool.tile([C, C], f32)
            make_identity(nc, ident[:])
            w_sb = cpool.tile([C, 2 * C], f32)
            nc.sync.dma_start(out=w_sb[:], in_=w[:])
            # wg = W2.T in bf16 (g needs no bias)
            t2 = psum.tile([C, C], f32, tag="wt", bufs=1)
            nc.tensor.transpose(out=t2[:], in_=w_sb[:, C : 2 * C], identity=ident[:])
            wg = cpool.tile([C, C], bf16)
            nc.vector.tensor_copy(out=wg[:], in_=t2[:])

            NSPLIT = 20
            g_a = pool.tile([P, NSPLIT, 2, C], bf16, bufs=1)
            g_b = pool.tile([P, NT - NSPLIT, 2, C], bf16, bufs=1)
            for b0 in range(0, NT, 2):  # 2 node tiles x 2 halves per PSUM tile
                nb = min(2, NT - b0)
                h8 = psum.tile([P, nb * 2, C], f32, tag=f"h{nb}", name=f"h8_{b0}", bufs=3 if nb == 2 else 1)
                for i in range(nb):
                    for s in range(2):
                        nc.tensor.matmul(
                            out=h8[:, 2 * i + s, :],
                            lhsT=xt[:, b0 + i, s, :],
                            rhs=wg[:],
                            start=True,
                            stop=True,
                        )
                if b0 + nb <= NSPLIT:
                    dst = g_a[:, b0 : b0 + nb, :, :]
                else:
                    dst = g_b[:, b0 - NSPLIT : b0 - NSPLIT + nb, :, :]
                # alternate copy engines so neither chain binds the pipeline
                eng = nc.vector.tensor_copy if (b0 // 2) % 2 == 0 else nc.scalar.copy
                eng(out=dst.rearrange("p t s c -> p (t s) c"), in_=h8[:])
                if b0 + nb == NSPLIT:  # overlap most of the store with the tail
                    nc.sync.dma_start(
                        out=gp[0 : NSPLIT * P, :].rearrange("(t p) c -> p t c", p=P),
                        in_=g_a[:],
                    )
            nc.sync.dma_start(
                out=gp[NSPLIT * P :, :].rearrange("(t p) c -> p t c", p=P),
                in_=g_b[:],
            )
    nc.compile()
    return nc


def _build_gather():
    import concourse.bacc as bacc
    import concourse.mybir as mybir
    from concourse.tile import TileContext
    from concourse.masks import make_identity

    nc = bacc.Bacc("TRN2", target_bir_lowering=False, debug=False)
    f32 = mybir.dt.float32
    bf16 = mybir.dt.bfloat16
    i16 = mybir.dt.int16
    u8 = mybir.dt.uint8
    # pair table: row r = [g_{2r} | g_{2r+1}] (256B); row NPAIR = sentinel
    gpair = nc.dram_tensor("gpair", [NPAIR + 1, 2 * C], bf16, kind="ExternalInput")
    # idx layout: batch-major, within batch positions wrapped 16-wide and
    # replicated to 128 partitions (dma_gather index convention)
    idx = nc.dram_tensor("idx", [P, IDXW], i16, kind="ExternalInput")
    msk = nc.dram_tensor("msk", [P, NT * 2 * DEG], u8, kind="ExternalInput")
    xta = nc.dram_tensor("xta", [C + 1, NSH_PAD], bf16, kind="ExternalInput")
    w = nc.dram_tensor("w", [C, 2 * C], f32, kind="ExternalInput")
    brow = nc.dram_tensor("brow", [1, C], f32, kind="ExternalInput")
    osh = nc.dram_tensor("osh", [NSH_PAD // 2, 2 * C], f32, kind="ExternalOutput")

    with TileContext(nc) as tc:
        with (
            tc.tile_pool(name="const", bufs=1) as cpool,
            tc.tile_pool(name="gat", bufs=3) as gpool,
            tc.tile_pool(name="psum", bufs=3, space="PSUM") as psum,
        ):
            idx_all = cpool.tile([P, IDXW], i16)
            # split so batch-0 desc-gen starts ~2.5us in, rest loads under it
            nc.sync.dma_start(out=idx_all[:, 0:512], in_=idx[:, 0:512])
            nc.sync.dma_start(out=idx_all[:, 512:], in_=idx[:, 512:])
            msk_all = cpool.tile([P, NT, 2 * DEG], u8)
            nc.sync.dma_start(
                out=msk_all[:], in_=msk[:, :].rearrange("p (t s) -> p t s", t=NT)
            )
            xt = cpool.tile([C + 1, NT, 2, P], bf16)
            nc.sync.dma_start(
                out=xt[:], in_=xta[:, :].rearrange("c (t s p) -> c t s p", s=2, p=P)
            )
            w_sb = cpool.tile([C, 2 * C], f32)
            nc.sync.dma_start(out=w_sb[:], in_=w[:])
            ident = cpool.tile([C, C], f32)
            make_identity(nc, ident[:])

            # wa = [(W1-W2).T ; b] in bf16 ([65, 64]); bias via xT ones row
            t1 = psum.tile([C, C], f32, tag="wt")
            t2 = psum.tile([C, C], f32, tag="wt")
            nc.tensor.transpose(out=t1[:], in_=w_sb[:, 0:C], identity=ident[:])
            nc.tensor.transpose(out=t2[:], in_=w_sb[:, C : 2 * C], identity=ident[:])
            wa_f = cpool.tile([C + 1, C], f32)
            nc.sync.dma_start(out=wa_f[C : C + 1, :], in_=brow[:])
            nc.vector.tensor_copy(out=wa_f[0:C, :], in_=t1[:])
            nc.vector.tensor_sub(out=wa_f[0:C, :], in0=wa_f[0:C, :], in1=t2[:])
            wa = cpool.tile([C + 1, C], bf16)
            nc.scalar.copy(out=wa[:], in_=wa_f[:])

            # A = x @ (W1-W2).T + b, recomputed on device in pair-major slots
            a_all = cpool.tile([P, NT, 2, C], f32)
            for b in range(NT):
                h4 = psum.tile([P, 2, C], f32, tag="h")
                for s in range(2):
                    nc.tensor.matmul(
                        out=h4[:, s, :],
                        lhsT=xt[:, b, s, :],
                        rhs=wa[:],
                        start=True,
                        stop=True,
                    )
                nc.scalar.copy(out=a_all[:, b, :, :], in_=h4[:])

            o_a = cpool.tile([P, 22, 2, C], f32)
            o_b = cpool.tile([P, 3, 2, C], f32)
            ioff = 0
            for b in range(NB):
                t0, nt_b = BATCHES[b]
                ni = nt_b * 2 * DEG * P  # gather positions
                gath = gpool.tile(
                    [P, nt_b * 2 * DEG, 2 * C], bf16, tag=f"gath{nt_b}",
                    name=f"gath_{b}",
                )
                nc.gpsimd.dma_gather(
                    out_ap=gath[:],
                    in_ap=gpair[:],
                    idxs_ap=idx_all[:, ioff : ioff + ni // 16],
                    num_idxs=ni,
                    num_idxs_reg=ni,
                    elem_size=2 * C,
                    transpose=False,
                    queue_num=0,
                    single_packet=False,
                )
                ioff += ni // 16
                # parity select in place: odd rows overwrite the even half
                nc.vector.copy_predicated(
                    out=gath[:, :, 0:C],
                    mask=msk_all[:, t0 : t0 + nt_b, :]
                    .rearrange("p t s -> p (t s)")
                    .to_broadcast([P, nt_b * 2 * DEG, C]),
                    data=gath[:, :, C : 2 * C],
                )
                # 4-round pairwise tree max: 16 slots -> 1 per segment
                cur = gath[:]
                nslots = nt_b * 2 * DEG
                for r in range(4):
                    nxt = gpool.tile(
                        [P, nslots // 2, C],
                        bf16,
                        tag=f"m{r}_{nt_b}",
                        name=f"m{r}_{nt_b}_{b}",
                    )
                    v = cur.rearrange("p (a two) c -> p a two c", two=2)
                    nc.vector.tensor_max(
                        out=nxt[:], in0=v[:, :, 0, 0:C], in1=v[:, :, 1, 0:C]
                    )
                    cur = nxt[:]
                    nslots //= 2
                # + A, relu, into the output accumulator
                s_sb = gpool.tile(
                    [P, nt_b * 2, C], f32, tag=f"s_{nt_b}", name=f"s_sb_{b}"
                )
                nc.vector.tensor_add(
                    out=s_sb[:],
                    in0=cur,
                    in1=a_all[:, t0 : t0 + nt_b, :, :].rearrange(
                        "p t s c -> p (t s) c"
                    ),
                )
                o_dst = (
                    o_a[:, t0 : t0 + nt_b, :, :]
                    if nt_b == 2
                    else o_b[:, t0 - 22 : t0 - 21, :, :]
                )
                nc.scalar.activation(
                    out=o_dst.rearrange("p t s c -> p (t s) c"),
                    in_=s_sb[:],
                    func=mybir.ActivationFunctionType.Relu,
                )
                if nt_b == 2 and t0 + nt_b == 22:
                    # tiles 0..21 store right when the gathers drain the DMA
                    nc.sync.dma_start(
                        out=osh[0 : 22 * P, :].rearrange("(t p) c -> p t c", p=P),
                        in_=o_a[:],
                    )
                elif nt_b == 1:
                    # per-tile stores chase the tail batches as they finish
                    nc.sync.dma_start(
                        out=osh[t0 * P : (t0 + 1) * P, :].rearrange(
                            "(t p) c -> p t c", p=P
                        ),
                        in_=o_b[:, t0 - 22 : t0 - 21, :, :],
                    )
    nc.compile()
    return nc


def _host_inputs(x, src, W, b):
    """Per-core host-side input staging (index/layout work only)."""
    bf16 = _bf16()
    W = W.astype(np.float32)
    brow = np.ascontiguousarray(b.astype(np.float32).reshape(1, C))
    ins1, ins2 = [], []
    for c in range(N_CORES):
        xs = np.zeros((NSH_PAD, C), dtype=np.float32)
        xs[:NSH] = x[c * NSH : (c + 1) * NSH]
        # pair-major column order: col j holds node 2*(128*(j//256) + j%128) + (j//128)%2
        t = np.arange(NSH_PAD) // 256
        s = (np.arange(NSH_PAD) // 128) % 2
        p = np.arange(NSH_PAD) % 128
        node = 2 * (128 * t + p) + s
        xta = np.ones((C + 1, NSH_PAD), dtype=np.float32)
        xta[:C] = xs[node].T
        xta = np.ascontiguousarray(xta.astype(bf16))

        # gather indices: per batch, position j covers (tile, half, k):
        #   part = j%128, slot = j//128 (0..nt*32), slot -> (tile_off, s, k)
        sc = np.full((NSH_PAD, DEG), -1, dtype=np.int64)
        sc[:NSH] = src[c * NSH * DEG : (c + 1) * NSH * DEG].reshape(NSH, DEG)
        idx = np.zeros((P, IDXW), dtype=np.int16)
        mskb = np.zeros((P, NT, 2 * DEG), dtype=np.uint8)
        ioff = 0
        for bb in range(NB):
            t0, nt_b = BATCHES[bb]
            ni = nt_b * 2 * DEG * P
            j = np.arange(ni)
            pj = j % 128
            slot = j // 128
            tile = t0 + slot // (2 * DEG)
            s_seg = (slot % (2 * DEG)) // DEG
            k = slot % DEG
            node_l = 2 * (128 * tile + pj) + s_seg
            sv = sc[node_l, k]
            pidx = np.where(sv >= 0, sv >> 1, NPAIR).astype(np.int16)
            par = np.where(sv >= 0, sv & 1, 0).astype(np.uint8)
            # idx wrap: position j -> [j%16, j//16], replicated down partitions
            a16 = np.swapaxes(pidx.reshape(ni // 16, 16), 0, 1)  # [16, ni/16]
            idx[:, ioff : ioff + ni // 16] = np.tile(a16, (8, 1))
            ioff += ni // 16
            # mask in dest layout [partition, tile, slot-within-tile]
            m = par.reshape(nt_b * 2 * DEG, P).T  # [p, slots]
            mskb[:, t0 : t0 + nt_b, :] = m.reshape(P, nt_b, 2 * DEG)
        msk = np.ascontiguousarray(mskb.reshape(P, NT * 2 * DEG))
        ins1.append({"xta": xta, "w": W})
        ins2.append(
            {"idx": np.ascontiguousarray(idx), "msk": msk, "xta": xta, "w": W,
             "brow": brow}
        )
    return ins1, ins2


def _unshard_out(r2):
    out = np.empty((N_NODES, C), dtype=np.float32)
    for c in range(N_CORES):
        o = r2.results[c]["osh"].reshape(NT, P, 2, C)
        o = np.transpose(o, (0, 1, 2, 3)).reshape(NT * P * 2, C)
        # row (t*128+p)*2+s = node 2*(128t+p)+s  -> already node order
        out[c * NSH : (c + 1) * NSH] = o[:NSH]
    return out


def _numpy_fallback(x, edge_index, W, b):
    src, dst = edge_index[0], edge_index[1]
    V1 = W[:, :C] - W[:, C:]
    V2 = W[:, C:]
    A = x @ V1.T + b
    g = x @ V2.T
    out = np.full((x.shape[0], C), -np.inf, dtype=np.float32)
    msg = np.maximum(A[dst] + g[src], 0.0)
    np.maximum.at(out, dst, msg)
    return np.where(np.isneginf(out), 0.0, out).astype(np.float32)


def _run_spmd(nc, in_maps):
    # the shared axon device occasionally reports a transient
    # NRT_EXEC_UNIT_UNRECOVERABLE on a cold first launch; retry once
    import time
    from concourse.bass_utils import run_bass_kernel_spmd

    try:
        return run_bass_kernel_spmd(nc, in_maps, core_ids=list(range(N_CORES)))
    except Exception:
        time.sleep(10.0)
        return run_bass_kernel_spmd(nc, in_maps, core_ids=list(range(N_CORES)))


def kernel(x, edge_index, edge_attr, W, b):
    x = np.ascontiguousarray(x, dtype=np.float32)
    edge_index = np.ascontiguousarray(edge_index, dtype=np.int32)
    W = np.ascontiguousarray(W, dtype=np.float32)
    b = np.ascontiguousarray(b, dtype=np.float32)

    expected_dst = np.repeat(np.arange(N_NODES, dtype=np.int32), DEG)
    if (
        x.shape != (N_NODES, C)
        or edge_index.shape != (2, N_NODES * DEG)
        or not np.array_equal(edge_index[1], expected_dst)
    ):
        return _numpy_fallback(x, edge_index, W, b)

    if "dense" not in _cache:
        _cache["dense"] = _build_dense()
    if "gather" not in _cache:
        _cache["gather"] = _build_gather()

    ins1, ins2 = _host_inputs(x, edge_index[0], W, b)

    # ---- Launch 1: per-core g table slices (pair layout) ----
    r1 = _run_spmd(_cache["dense"], ins1)

    bf16 = _bf16()
    gpair = np.empty((NPAIR + 1, 2 * C), dtype=bf16)
    for c in range(N_CORES):
        gp = r1.results[c]["gp"].reshape(NT, P, 2 * C)
        # row t*128+p = local pair t*128+p -> global pair c*3125 + ...
        gpair[c * (NSH // 2) : (c + 1) * (NSH // 2)] = gp.reshape(
            NT * P, 2 * C
        )[: NSH // 2]
    gpair[NPAIR] = SENT

    # ---- Launch 2: edge gather + segment max ----
    for c in range(N_CORES):
        ins2[c]["gpair"] = gpair
    r2 = _run_spmd(_cache["gather"], ins2)

    out = _unshard_out(r2)
    _cache["last_results"] = (r1, r2)
    return out
